# revision 35
# baseline (speedup 1.0000x reference)
"""Decoder block (8-head causal attention + FFN + 2x layernorm) on 8 trn2 cores.

Problem: x (4, 2048, 512) fp32; per-head Wq/Wk/Wv (8, 512, 64); Wo (512, 512);
FFN 512->2048->512; causal mask; two post-residual layernorms.

Sharding (uniform SPMD program, 8 cores): core c -> (batch n = c//2,
head-half s = c%2). Each core computes Q/K/V for its 4 heads over the full
2048-token sequence of its batch and causal attention for all 2048 queries.
Each core computes its Wo partial (contraction over its 256 channels) for
all rows in bf16; two pairwise ReduceScatters sum the partials and hand each
core its own 1024 rows. Each core then runs residual+LN1, FFN and
residual+LN2 for its rows. Host reassembles.

v2 restructure vs v1 baseline (281us):
 - input DMAs ordered by first use (xT/wk/wq/wv first, w1/w2/xbo last) so
   the first matmul starts ~8us in instead of ~30us.
 - scores packed 2 heads per step via row-tiled concurrent matmuls (K=64,
   tile rows 0-63 = even head, 64-127 = odd head) -> ~2x score PE time.
 - qT is one tile per channel-chunk (even head rows 0:64, odd 64:128) --
   the natural PSUM layout; halves the q drain calls and SBUF.
 - diagonal blocks packed 2-per-psum-tile per head half: fewer, larger exp
   calls with no stale columns.
 - ACT does exp only (plus phase-3 relu/sqrt after all exps -> 2 table
   loads); k/q/v drains on DVE; triangle masks on DVE (127ns vs 406 gpsimd);
   LN gain/bias on gpsimd; denominator reciprocal reads PSUM directly
   ([1,1024] covers both heads, one DRAM round-trip broadcast per group).
 - Wo partial PSUM shares the score pool slots (PSUM = 2x[128,1024] scores
   + 2x[128,1024] att accumulators = exactly 8 banks).
 - phase 3 ordered so the half-1 LN chain is emitted before half-0's LN2
   (no DVE FIFO head-block while RS B is in flight).

All matmuls bf16 with fp32 PSUM accumulation; softmax without
max-subtraction; denominator via ones-column in V.
"""

import sys

sys.path.insert(0, "/opt/trn_rl_repo")

import numpy as np
import ml_dtypes

import concourse.bacc as bacc
import concourse.bass as bass
import concourse.mybir as mybir
import concourse.tile as tile
from concourse import bass_utils, masks

F32 = mybir.dt.float32
BF16 = mybir.dt.bfloat16
AF = mybir.ActivationFunctionType
ALU = mybir.AluOpType

N, K, D, H, F = 4, 2048, 512, 8, 2048
Dh = D // H          # 64
HH = H // 2          # 4 local heads per core
E = HH * Dh          # 256 local attention channels
EPS = 1e-10
N_CORES = 8
OWN = K // 2         # 1024 rows per core after the exchange

_CACHE = {}
DEBUG = False


def _build():
    nc = bacc.Bacc("TRN2", target_bir_lowering=False, debug=False,
                   num_devices=N_CORES)

    xt_d = nc.dram_tensor("xt", [D, K], BF16, kind="ExternalInput")
    xbo_d = nc.dram_tensor("xbo", [OWN, D], F32, kind="ExternalInput")
    wq_d = nc.dram_tensor("wq", [D, E], BF16, kind="ExternalInput")
    wk_d = nc.dram_tensor("wk", [D, E], BF16, kind="ExternalInput")
    wv_d = nc.dram_tensor("wv", [D, E], BF16, kind="ExternalInput")
    bq_d = nc.dram_tensor("bqc", [128, 2], F32, kind="ExternalInput")
    bk_d = nc.dram_tensor("bkc", [128, 2], F32, kind="ExternalInput")
    bv_d = nc.dram_tensor("bvr", [1, E], F32, kind="ExternalInput")
    wo_d = nc.dram_tensor("wo", [E, D], BF16, kind="ExternalInput")
    w1_d = nc.dram_tensor("w1", [D, F], BF16, kind="ExternalInput")
    b1_d = nc.dram_tensor("b1c", [128, 16], F32, kind="ExternalInput")
    w2_d = nc.dram_tensor("w2", [F, D], BF16, kind="ExternalInput")
    b2_d = nc.dram_tensor("b2r", [1, D], F32, kind="ExternalInput")
    g1_d = nc.dram_tensor("g1r", [1, D], F32, kind="ExternalInput")
    be1_d = nc.dram_tensor("be1r", [1, D], F32, kind="ExternalInput")
    g2_d = nc.dram_tensor("g2r", [1, D], F32, kind="ExternalInput")
    be2_d = nc.dram_tensor("be2r", [1, D], F32, kind="ExternalInput")
    out_d = nc.dram_tensor("out", [OWN, D], F32, kind="ExternalOutput")
    if DEBUG:
        dbg_kt = nc.dram_tensor("dbg_kt", [128, K], BF16,
                                kind="ExternalOutput")
        dbg_qt = nc.dram_tensor("dbg_qt", [128, K], BF16,
                                kind="ExternalOutput")
        dbg_va = nc.dram_tensor("dbg_va", [128, HH * 128], BF16,
                                kind="ExternalOutput")
        dbg_ac = nc.dram_tensor("dbg_ac", [2, 128, K], BF16,
                                kind="ExternalOutput")
        dbg_rs = nc.dram_tensor("dbg_rs", [OWN, D], BF16,
                                kind="ExternalOutput")
        dbg_h1 = nc.dram_tensor("dbg_h1", [128, D], F32,
                                kind="ExternalOutput")

    def bcast(dram, npart, n):
        return bass.AP(tensor=dram, offset=0, ap=[[0, npart], [1, n]])

    with tile.TileContext(nc) as tc:
        import contextlib
        stack = contextlib.ExitStack()
        with stack:
            singles = stack.enter_context(tc.tile_pool(name="singles", bufs=1))
            dram = stack.enter_context(
                tc.tile_pool(name="dram", bufs=1, space="DRAM"))
            drp = stack.enter_context(
                tc.tile_pool(name="drp", bufs=4, space="DRAM"))

            # ---- input DMAs in first-use order -------------------------
            pw = stack.enter_context(tc.tile_pool(name="pw", bufs=1))
            xT = [pw.tile([128, K], BF16, name=f"xT{i}") for i in range(4)]
            for kb in range(4):
                for dc in range(4):
                    nc.sync.dma_start(
                        out=xT[dc][:, kb * 512:(kb + 1) * 512],
                        in_=xt_d[dc * 128:(dc + 1) * 128,
                                 kb * 512:(kb + 1) * 512])
            wk_sb = [pw.tile([128, E], BF16, name=f"wk{i}") for i in range(4)]
            for dc in range(4):
                nc.sync.dma_start(out=wk_sb[dc],
                                  in_=wk_d[dc * 128:(dc + 1) * 128, :])
            wq_sb = [pw.tile([128, E], BF16, name=f"wq{i}") for i in range(4)]
            for dc in range(4):
                nc.sync.dma_start(out=wq_sb[dc],
                                  in_=wq_d[dc * 128:(dc + 1) * 128, :])
            wv_sb = [pw.tile([128, E], BF16, name=f"wv{i}") for i in range(4)]
            for dc in range(4):
                nc.sync.dma_start(out=wv_sb[dc],
                                  in_=wv_d[dc * 128:(dc + 1) * 128, :])
            bk_col = singles.tile([128, 2], F32)
            nc.sync.dma_start(out=bk_col, in_=bk_d[:, :])
            bq_col = singles.tile([128, 2], F32)
            nc.sync.dma_start(out=bq_col, in_=bq_d[:, :])
            b1_col = singles.tile([128, 16], F32)
            nc.sync.dma_start(out=b1_col, in_=b1_d[:, :])
            wo_sb = [pw.tile([128, D], BF16, name=f"wo{i}") for i in range(2)]
            for cc in range(2):
                nc.sync.dma_start(out=wo_sb[cc],
                                  in_=wo_d[cc * 128:(cc + 1) * 128, :])
            # late-use inputs last (needed only in phase 3)
            w1_sb = [pw.tile([128, F], BF16, name=f"w1_{i}") for i in range(4)]
            for dc in range(4):
                for fb in range(4):
                    nc.sync.dma_start(
                        out=w1_sb[dc][:, fb * 512:(fb + 1) * 512],
                        in_=w1_d[dc * 128:(dc + 1) * 128,
                                 fb * 512:(fb + 1) * 512])
            w2_sb = [pw.tile([128, D], BF16, name=f"w2_{i}") for i in range(16)]
            for fc in range(16):
                nc.sync.dma_start(out=w2_sb[fc],
                                  in_=w2_d[fc * 128:(fc + 1) * 128, :])
            xbo_sb = [pw.tile([128, D], F32, name=f"xbo{i}") for i in range(8)]
            for qt in range(8):
                nc.sync.dma_start(out=xbo_sb[qt],
                                  in_=xbo_d[qt * 128:(qt + 1) * 128, :])

            # broadcasts (gpsimd software DMA handles 0-stride partitions)
            bv_bc = singles.tile([128, E], F32)
            nc.gpsimd.dma_start(out=bv_bc, in_=bcast(bv_d, 128, E))
            g1_bc = singles.tile([128, D], F32)
            nc.gpsimd.dma_start(out=g1_bc, in_=bcast(g1_d, 128, D))
            beb2_bc = singles.tile([128, D], F32)
            nc.gpsimd.dma_start(out=beb2_bc, in_=bcast(be1_d, 128, D))
            g2_bc = singles.tile([128, D], F32)
            nc.gpsimd.dma_start(out=g2_bc, in_=bcast(g2_d, 128, D))
            be2_bc = singles.tile([128, D], F32)
            nc.gpsimd.dma_start(out=be2_bc, in_=bcast(be2_d, 128, D))

            # ---- static tiles ------------------------------------------
            ident = singles.tile([128, 128], F32)
            masks.make_identity(nc, ident[:])
            tri01 = singles.tile([128, 128], BF16)
            nc.gpsimd.memset(tri01, 1.0)
            # keep 1.0 where q - k >= 0 (partition = key, free = query)
            nc.gpsimd.affine_select(
                out=tri01, in_=tri01, compare_op=ALU.is_ge,
                fill=0.0, base=0, pattern=[[1, 128]], channel_multiplier=-1)
            eps_t = singles.tile([128, 1], F32)
            nc.vector.memset(eps_t, EPS)

            # ---- persistent activation tensors -------------------------
            kt_pool = stack.enter_context(tc.tile_pool(name="kt", bufs=1))
            qt_pool = stack.enter_context(tc.tile_pool(name="qt", bufs=1))
            va_pool = stack.enter_context(tc.tile_pool(name="va", bufs=1))
            ac_pool = stack.enter_context(tc.tile_pool(name="ac", bufs=1))
            kT = [kt_pool.tile([128, K], BF16, name=f"kT{i}") for i in range(2)]
            qT = [qt_pool.tile([128, K], BF16, name=f"qT{i}") for i in range(2)]
            va = [va_pool.tile([128, HH, 128], BF16, name=f"va{i}")
                  for i in range(K // 128)]
            ac = [ac_pool.tile([128, K], BF16, name=f"ac{i}") for i in range(2)]
            for kt_i in range(K // 128):
                nc.gpsimd.memset(va[kt_i][:, :, Dh:128], 0.0)
                nc.gpsimd.memset(va[kt_i][:, :, Dh:Dh + 1], 1.0)

            h1_pool = stack.enter_context(tc.tile_pool(name="h1", bufs=1))
            h1 = [h1_pool.tile([128, D], F32, name=f"h1_{i}") for i in range(8)]
            h1t_pool = stack.enter_context(tc.tile_pool(name="h1t", bufs=1))
            h1T = h1t_pool.tile([128, 4 * OWN], BF16, name="h1T")
            lnp = stack.enter_context(tc.tile_pool(name="lnp", bufs=4))

            # ---------------- phase 1: projections ----------------------
            with tc.tile_pool(name="ps_p", bufs=4, space="PSUM") as ps_p:
                for cc in range(2):
                    for kb in range(4):
                        pp = ps_p.tile([128, 512], F32, name="pp")
                        for dc in range(4):
                            nc.tensor.matmul(
                                pp[:],
                                wk_sb[dc][:, cc * 128:(cc + 1) * 128],
                                xT[dc][:, kb * 512:(kb + 1) * 512],
                                start=(dc == 0), stop=(dc == 3))
                        nc.vector.tensor_scalar(
                            out=kT[cc][:, kb * 512:(kb + 1) * 512],
                            in0=pp[:], scalar1=bk_col[:, cc:cc + 1],
                            scalar2=None, op0=ALU.add)
                for cc in range(2):
                    for kb in range(4):
                        pp = ps_p.tile([128, 512], F32, name="pp")
                        for dc in range(4):
                            nc.tensor.matmul(
                                pp[:],
                                wq_sb[dc][:, cc * 128:(cc + 1) * 128],
                                xT[dc][:, kb * 512:(kb + 1) * 512],
                                start=(dc == 0), stop=(dc == 3))
                        nc.vector.tensor_scalar(
                            out=qT[cc][:, kb * 512:(kb + 1) * 512],
                            in0=pp[:], scalar1=bq_col[:, cc:cc + 1],
                            scalar2=None, op0=ALU.add)
                # v rows (4 local heads at once); ones column pre-memset
                for kt_i in range(K // 128):
                    vp = ps_p.tile([128, E], F32, name="vp")
                    for dc in range(4):
                        nc.tensor.matmul(
                            vp[:],
                            xT[dc][:, kt_i * 128:(kt_i + 1) * 128],
                            wv_sb[dc][:], start=(dc == 0), stop=(dc == 3))
                    nc.vector.tensor_add(
                        out=va[kt_i][:, :, 0:Dh],
                        in0=vp[:].rearrange("p (h e) -> p h e", h=HH),
                        in1=bv_bc[:].rearrange("p (h e) -> p h e", h=HH))

            # ---------------- phase 2: attention + Wo partials + RS -----
            rsA_in = dram.tile([1024, D], BF16, name="rsA_in")
            rsB_in = dram.tile([1024, D], BF16, name="rsB_in")
            rsA_out = dram.tile([512, D], BF16, name="rsA_out")
            rsB_out = dram.tile([512, D], BF16, name="rsB_out")

            # diag packing: jp0 tile holds m0 (w=512) @ col 0 and
            # m1 (w=384) @ col 512; jp1 tile holds m2 (w=256) @ col 0 and
            # m3 (w=128) @ col 256.
            DIAG = [  # (m, colbase, width)
                [(0, 0, 512), (1, 512, 384)],
                [(2, 0, 256), (3, 256, 128)],
            ]

            def attn_pair(cc, qb, ps_s, ps_att, expp, bcp):
                """Causal attention for heads (2cc, 2cc+1), query block qb."""
                qs = qb * 512
                att = ps_att.tile([128, 1024], F32, name="att")
                n_av = [0, 0]

                def av(hh, kb, lo, exp_ap, stop):
                    # two interleaved accumulation groups (hh=0 cols 0:512,
                    # hh=1 cols 512:1024) share the att tile
                    nc.tensor.matmul(
                        att[:, hh * 512 + lo:hh * 512 + 512],
                        va[kb][:, 2 * cc + hh, :], exp_ap,
                        start=(n_av[hh] == 0), stop=stop,
                        skip_group_check=True)
                    n_av[hh] += 1

                # full key blocks: one [128,1024] tile = 1 kb x 2 heads
                for kb in range(4 * qb):
                    s2 = ps_s.tile([128, 1024], F32, name="s2")
                    for hh in range(2):
                        nc.tensor.matmul(
                            s2[:, hh * 512:(hh + 1) * 512],
                            kT[cc][hh * 64:(hh + 1) * 64,
                                   kb * 128:(kb + 1) * 128],
                            qT[cc][hh * 64:(hh + 1) * 64, qs:qs + 512],
                            start=True, stop=True)
                    expT = expp.tile([128, 1024], BF16, name="expT")
                    nc.scalar.activation(out=expT[:], in_=s2[:],
                                         func=AF.Exp, scale=0.125)
                    for hh in range(2):
                        av(hh, kb, 0, expT[:, hh * 512:(hh + 1) * 512],
                           stop=False)
                # diagonal: per jp, one tile per head half (no stale cols)
                for jp in range(2):
                    dt = [ps_s.tile([128, 1024], F32, name="s2")
                          for _ in range(2)]
                    for (m, cb, w) in DIAG[jp]:
                        kb = 4 * qb + m
                        lo = m * 128
                        for hh in range(2):
                            nc.tensor.matmul(
                                dt[hh][:, cb:cb + w],
                                kT[cc][hh * 64:(hh + 1) * 64,
                                       kb * 128:(kb + 1) * 128],
                                qT[cc][hh * 64:(hh + 1) * 64,
                                       qs + lo:qs + 512],
                                start=True, stop=True)
                    tot = sum(w for (_, _, w) in DIAG[jp])
                    expd = [expp.tile([128, 1024], BF16, name="expT")
                            for _ in range(2)]
                    for hh in range(2):
                        nc.scalar.activation(out=expd[hh][:, 0:tot],
                                             in_=dt[hh][:, 0:tot],
                                             func=AF.Exp, scale=0.125)
                    # zero the still-masked triangle (k > q) on DVE
                    for hh in range(2):
                        for (m, cb, w) in DIAG[jp]:
                            nc.vector.tensor_mul(
                                out=expd[hh][:, cb:cb + 128],
                                in0=expd[hh][:, cb:cb + 128], in1=tri01[:])
                    for (m, cb, w) in DIAG[jp]:
                        kb = 4 * qb + m
                        lo = m * 128
                        last = (jp == 1 and m == 3)
                        for hh in range(2):
                            av(hh, kb, lo, expd[hh][:, cb:cb + w], stop=last)
                # normalize: drain att to SBUF first so its PSUM slot frees
                # immediately -- the DRAM round-trip broadcast below can
                # stall >10us when a ReduceScatter hogs the DMA queues,
                # which otherwise blocks the next group's AV accumulation.
                # (reciprocal_approx_fast only works at base partition 0, so
                # the den row is copied down to partition 0 from SBUF.)
                araw = bcp.tile([128, 1024], F32, name="araw")
                nc.vector.tensor_copy(out=araw[:], in_=att[:])
                den = bcp.tile([1, 1024], F32, name="den")
                nc.vector.tensor_copy(out=den[:], in_=araw[64:65, :])
                rec = bcp.tile([1, 1024], F32, name="rec")
                nc.vector.reciprocal_approx_fast(out=rec[:], in_=den[:])
                rec_dr = drp.tile([1, 1024], F32, name="rec_dr")
                nc.sync.dma_start(out=rec_dr[:], in_=rec[:])
                bc_sb = bcp.tile([64, 1024], F32, name="bc_sb")
                nc.sync.dma_start(out=bc_sb[:], in_=bass.AP(
                    tensor=rec_dr[:].tensor, offset=rec_dr[:].offset,
                    ap=[[0, 64], [1, 1024]]))
                for hh in range(2):
                    nc.vector.tensor_mul(
                        out=ac[cc][hh * 64:(hh + 1) * 64, qs:qs + 512],
                        in0=araw[0:64, hh * 512:(hh + 1) * 512],
                        in1=bc_sb[:, hh * 512:(hh + 1) * 512])

            def wo_partial(qb, dest, row0, ps_s, wop):
                """o_part[qb] = ac[:, qb]^T @ wo -> bf16 -> rs_in rows."""
                for half in range(2):
                    o_ps = ps_s.tile([128, 1024], F32, name="s2")
                    for g in range(2):
                        qt2 = half * 2 + g
                        for cc in range(2):
                            nc.tensor.matmul(
                                o_ps[:, g * 512:(g + 1) * 512],
                                ac[cc][:, qb * 512 + qt2 * 128:
                                       qb * 512 + (qt2 + 1) * 128],
                                wo_sb[cc][:], start=(cc == 0), stop=(cc == 1))
                    o_sb = wop.tile([128, 1024], BF16, name="o_sb")
                    nc.vector.tensor_copy(out=o_sb[:], in_=o_ps[:])
                    for g in range(2):
                        qt2 = half * 2 + g
                        nc.gpsimd.dma_start(
                            out=dest[row0 + qt2 * 128:row0 + (qt2 + 1) * 128,
                                     :],
                            in_=o_sb[:, g * 512:(g + 1) * 512])

            RG = [[0, 1], [2, 3], [4, 5], [6, 7]]
            # qb order (0,2,1,3); RS A = [qb0|qb2] launches mid-attention,
            # RS B = [qb1|qb3] at the end.
            wo_args = {0: (rsA_in, 0), 2: (rsA_in, 512),
                       1: (rsB_in, 0), 3: (rsB_in, 512)}
            with tc.tile_pool(name="ps_s", bufs=2, space="PSUM") as ps_s, \
                 tc.tile_pool(name="ps_att", bufs=2, space="PSUM") as ps_att, \
                 tc.tile_pool(name="expp", bufs=3) as expp, \
                 tc.tile_pool(name="bcp", bufs=2) as bcp, \
                 tc.tile_pool(name="wop", bufs=3) as wop:
                order = [0, 2, 1, 3]
                for qi, qb in enumerate(order):
                    attn_pair(0, qb, ps_s, ps_att, expp, bcp)
                    if qi >= 1:
                        pqb = order[qi - 1]
                        dest, row0 = wo_args[pqb]
                        wo_partial(pqb, dest, row0, ps_s, wop)
                    attn_pair(1, qb, ps_s, ps_att, expp, bcp)
                    if qi == 2:
                        nc.gpsimd.collective_compute(
                            "ReduceScatter", ALU.add, replica_groups=RG,
                            ins=[rsA_in[:]], outs=[rsA_out[:]])
                wo_partial(3, rsB_in, 512, ps_s, wop)
                nc.gpsimd.collective_compute(
                    "ReduceScatter", ALU.add, replica_groups=RG,
                    ins=[rsB_in[:]], outs=[rsB_out[:]])
                if DEBUG:
                    nc.sync.dma_start(out=dbg_kt[:, :], in_=kT[0][:])
                    nc.sync.dma_start(out=dbg_qt[:, :], in_=qT[0][:])
                    nc.sync.dma_start(out=dbg_va[:, :],
                                      in_=va[0][:].rearrange(
                                          "p h e -> p (h e)"))
                    for cc in range(2):
                        nc.sync.dma_start(out=dbg_ac[cc, :, :], in_=ac[cc][:])
                    nc.sync.dma_start(out=dbg_rs[0:512, :], in_=rsA_out[:])
                    nc.sync.dma_start(out=dbg_rs[512:1024, :],
                                      in_=rsB_out[:])

            # ---------------- phase 3: Wo + LN1 + FFN + LN2 -------------
            def layer_norm_core(pre, dst):
                stats = lnp.tile([128, 6], F32, name="ln_stats")
                nc.vector.bn_stats(out=stats[:], in_=pre[:])
                mv = lnp.tile([128, 2], F32, name="ln_mv")
                nc.vector.bn_aggr(out=mv[:], in_=stats[:])
                rstd = lnp.tile([128, 1], F32, name="ln_rstd")
                nc.scalar.activation(out=rstd[:], in_=mv[:, 1:2],
                                     func=AF.Sqrt, bias=eps_t[:])
                nc.vector.reciprocal(out=rstd[:], in_=rstd[:])
                nc.vector.tensor_scalar(
                    out=dst, in0=pre[:], scalar1=mv[:, 0:1],
                    scalar2=rstd[:], op0=ALU.subtract, op1=ALU.mult)

            def wo_ln1(half, src, orp):
                """rs_out read + residual + LN1 for rows of this half."""
                for qt2 in range(4):
                    qt = half * 4 + qt2
                    o_rs = orp.tile([128, D], BF16, name="o_rs")
                    nc.sync.dma_start(
                        out=o_rs,
                        in_=src[qt2 * 128:(qt2 + 1) * 128, :])
                    pre = lnp.tile([128, D], F32, name="ln_pre")
                    nc.vector.tensor_add(out=pre[:], in0=o_rs[:],
                                         in1=xbo_sb[qt][:])
                    layer_norm_core(pre, h1[qt][:])

            def transp(half, ps_aux):
                """transpose n1 into h1T, then fold g1/beb2 into h1."""
                for qt2 in range(4):
                    qt = half * 4 + qt2
                    trp = ps_aux.tile([128, D], F32, name="aux")
                    for dc in range(4):
                        nc.tensor.transpose(
                            trp[:, dc * 128:(dc + 1) * 128],
                            h1[qt][:, dc * 128:(dc + 1) * 128], ident[:])
                    nc.vector.tensor_copy(
                        out=h1T[:].rearrange("p (dc c) -> p dc c", dc=4)
                            [:, :, qt * 128:(qt + 1) * 128],
                        in_=trp[:].rearrange("p (dc c) -> p dc c", dc=4))
                for qt2 in range(4):
                    qt = half * 4 + qt2
                    nc.gpsimd.tensor_mul(out=h1[qt][:], in0=h1[qt][:],
                                         in1=g1_bc[:])
                    nc.gpsimd.tensor_add(out=h1[qt][:], in0=h1[qt][:],
                                         in1=beb2_bc[:])

            def ffn1(half, ps_f1, fap, fa_out):
                for fc in range(16):
                    fp_ps = ps_f1.tile([128, 512], F32, name="fp_ps")
                    for dc in range(4):
                        nc.tensor.matmul(
                            fp_ps[:],
                            w1_sb[dc][:, fc * 128:(fc + 1) * 128],
                            h1T[:, dc * OWN + half * 512:
                                dc * OWN + (half + 1) * 512],
                            start=(dc == 0), stop=(dc == 3))
                    fa_t = fap.tile([128, 512], BF16, name=f"fa{fc}")
                    nc.scalar.activation(out=fa_t[:], in_=fp_ps[:],
                                         func=AF.Relu,
                                         bias=b1_col[:, fc:fc + 1])
                    fa_out.append(fa_t)

            def ffn2(half, ps_f2, fa, outp):
                for qt2 in range(4):
                    qt = half * 4 + qt2
                    ff2_ps = ps_f2.tile([128, D], F32, name="ff2")
                    for fc in range(16):
                        nc.tensor.matmul(
                            ff2_ps[:],
                            fa[fc][:, qt2 * 128:(qt2 + 1) * 128],
                            w2_sb[fc][:], start=(fc == 0), stop=(fc == 15))
                    # h1 already holds n1*g1 + (be1+b2)
                    pre = lnp.tile([128, D], F32, name="ln_pre")
                    nc.vector.tensor_add(out=pre[:], in0=ff2_ps[:],
                                         in1=h1[qt][:])
                    out_sb = outp.tile([128, D], F32, name="out_sb")
                    layer_norm_core(pre, out_sb[:])
                    nc.gpsimd.tensor_mul(out=out_sb[:], in0=out_sb[:],
                                         in1=g2_bc[:])
                    nc.gpsimd.tensor_add(out=out_sb[:], in0=out_sb[:],
                                         in1=be2_bc[:])
                    nc.sync.dma_start(
                        out=out_d[qt * 128:(qt + 1) * 128, :], in_=out_sb[:])

            with tc.tile_pool(name="ps_aux", bufs=2, space="PSUM") as ps_aux, \
                 tc.tile_pool(name="ps_f1", bufs=3, space="PSUM") as ps_f1, \
                 tc.tile_pool(name="ps_f2", bufs=2, space="PSUM") as ps_f2, \
                 tc.tile_pool(name="fap", bufs=1) as fap, \
                 tc.tile_pool(name="orp", bufs=3) as orp, \
                 tc.tile_pool(name="outp", bufs=2) as outp:
                fa0, fa1 = [], []
                wo_ln1(0, rsA_out, orp)
                if DEBUG:
                    nc.sync.dma_start(out=dbg_h1[:, :], in_=h1[0][:])
                transp(0, ps_aux)
                ffn1(0, ps_f1, fap, fa0)
                # half-1 LN chain emitted before half-0's LN2 so the DVE
                # order matches readiness (RS B lands before ffn2(0) ends).
                wo_ln1(1, rsB_out, orp)
                ffn2(0, ps_f2, fa0, outp)
                transp(1, ps_aux)
                ffn1(1, ps_f1, fap, fa1)
                ffn2(1, ps_f2, fa1, outp)

    nc.compile()
    return nc


def _get_nc():
    if "nc" not in _CACHE:
        _CACHE["nc"] = _build()
    return _CACHE["nc"]


def _make_in_maps(x, Wq, bq, Wk, bk, Wv, bv, Wo, bo, W1, b1, W2, b2, g1, be1,
                  g2, be2):
    bf = ml_dtypes.bfloat16
    x = np.ascontiguousarray(np.asarray(x, dtype=np.float32))
    Wq, Wk, Wv = (np.asarray(w, np.float32) for w in (Wq, Wk, Wv))
    bo = np.asarray(bo, np.float32)
    g1f = np.asarray(g1, np.float32)
    be1f = np.asarray(be1, np.float32)
    w1f = np.asarray(W1, np.float32)
    w1b = np.ascontiguousarray((g1f[:, None] * w1f).astype(bf))
    b1f = np.asarray(b1, np.float32) + be1f @ w1f
    w2b = np.ascontiguousarray(np.asarray(W2, np.float32).astype(bf))
    wof = np.asarray(Wo, np.float32)
    b1c = np.ascontiguousarray(b1f.reshape(16, 128).T)
    in_maps = []
    for c in range(N_CORES):
        n, s = divmod(c, 2)
        hsel = slice(HH * s, HH * s + HH)
        in_maps.append({
            "xt": np.ascontiguousarray(x[n].T.astype(bf)),
            "xbo": np.ascontiguousarray(x[n, OWN * s:OWN * s + OWN] + bo),
            "wq": np.ascontiguousarray(
                Wq[hsel].transpose(1, 0, 2).reshape(D, E).astype(bf)),
            "wk": np.ascontiguousarray(
                Wk[hsel].transpose(1, 0, 2).reshape(D, E).astype(bf)),
            "wv": np.ascontiguousarray(
                Wv[hsel].transpose(1, 0, 2).reshape(D, E).astype(bf)),
            "bqc": np.ascontiguousarray(
                np.asarray(bq, np.float32)[hsel].reshape(2, 128).T),
            "bkc": np.ascontiguousarray(
                np.asarray(bk, np.float32)[hsel].reshape(2, 128).T),
            "bvr": np.ascontiguousarray(
                np.asarray(bv, np.float32)[hsel]).reshape(1, E),
            "wo": np.ascontiguousarray(wof[E * s:E * s + E].astype(bf)),
            "w1": w1b,
            "b1c": b1c,
            "w2": w2b,
            "b2r": np.asarray(b2, np.float32).reshape(1, D),
            # beb2 rides in the be1r slot: residual bias be1 + b2
            "g1r": np.asarray(g1, np.float32).reshape(1, D),
            "be1r": (be1f + np.asarray(b2, np.float32)).reshape(1, D),
            "g2r": np.asarray(g2, np.float32).reshape(1, D),
            "be2r": np.asarray(be2, np.float32).reshape(1, D),
        })
    return in_maps


def kernel(x, Wq, bq, Wk, bk, Wv, bv, Wo, bo, W1, b1, W2, b2, g1, be1, g2,
           be2, mask=None, **_unused):
    nc = _get_nc()
    in_maps = _make_in_maps(x, Wq, bq, Wk, bk, Wv, bv, Wo, bo, W1, b1, W2, b2,
                            g1, be1, g2, be2)
    res = bass_utils.run_bass_kernel_spmd(
        nc, in_maps, core_ids=list(range(N_CORES)))
    y = np.empty((N, K, D), np.float32)
    for c in range(N_CORES):
        n, s = divmod(c, 2)
        y[n, OWN * s:OWN * s + OWN] = res.results[c]["out"]
    return y


def kernel_timed(x, Wq, bq, Wk, bk, Wv, bv, Wo, bo, W1, b1, W2, b2, g1, be1,
                 g2, be2, mask=None, trace_cores=None, **_unused):
    """Run with NTFF tracing; returns BassKernelResults (exec_time_ns etc)."""
    nc = _get_nc()
    in_maps = _make_in_maps(x, Wq, bq, Wk, bk, Wv, bv, Wo, bo, W1, b1, W2, b2,
                            g1, be1, g2, be2)
    if trace_cores is None:
        trace_cores = list(range(N_CORES))
    return bass_utils.run_bass_kernel_spmd(
        nc, in_maps, core_ids=list(range(N_CORES)), trace=True,
        trace_cores=trace_cores)


# revision 36
# speedup vs baseline: 1.0065x; 1.0065x over previous
"""Decoder block (8-head causal attention + FFN + 2x layernorm) on 8 trn2 cores.

Problem: x (4, 2048, 512) fp32; per-head Wq/Wk/Wv (8, 512, 64); Wo (512, 512);
FFN 512->2048->512; causal mask; two post-residual layernorms.

Sharding (uniform SPMD program, 8 cores): core c -> (batch n = c//2,
head-half s = c%2). Each core computes Q/K/V for its 4 heads over the full
2048-token sequence of its batch and causal attention for all 2048 queries.
Each core computes its Wo partial (contraction over its 256 channels) for
all rows in bf16; two pairwise ReduceScatters sum the partials and hand each
core its own 1024 rows. Each core then runs residual+LN1, FFN and
residual+LN2 for its rows. Host reassembles.

v2 restructure vs v1 baseline (281us):
 - input DMAs ordered by first use (xT/wk/wq/wv first, w1/w2/xbo last) so
   the first matmul starts ~8us in instead of ~30us.
 - scores packed 2 heads per step via row-tiled concurrent matmuls (K=64,
   tile rows 0-63 = even head, 64-127 = odd head) -> ~2x score PE time.
 - qT is one tile per channel-chunk (even head rows 0:64, odd 64:128) --
   the natural PSUM layout; halves the q drain calls and SBUF.
 - diagonal blocks packed 2-per-psum-tile per head half: fewer, larger exp
   calls with no stale columns.
 - ACT does exp only (plus phase-3 relu/sqrt after all exps -> 2 table
   loads); k/q/v drains on DVE; triangle masks on DVE (127ns vs 406 gpsimd);
   LN gain/bias on gpsimd; denominator reciprocal reads PSUM directly
   ([1,1024] covers both heads, one DRAM round-trip broadcast per group).
 - Wo partial PSUM shares the score pool slots (PSUM = 2x[128,1024] scores
   + 2x[128,1024] att accumulators = exactly 8 banks).
 - phase 3 ordered so the half-1 LN chain is emitted before half-0's LN2
   (no DVE FIFO head-block while RS B is in flight).

All matmuls bf16 with fp32 PSUM accumulation; softmax without
max-subtraction; denominator via ones-column in V.
"""

import sys

sys.path.insert(0, "/opt/trn_rl_repo")

import numpy as np
import ml_dtypes

import concourse.bacc as bacc
import concourse.bass as bass
import concourse.mybir as mybir
import concourse.tile as tile
from concourse import bass_utils, masks

F32 = mybir.dt.float32
BF16 = mybir.dt.bfloat16
AF = mybir.ActivationFunctionType
ALU = mybir.AluOpType

N, K, D, H, F = 4, 2048, 512, 8, 2048
Dh = D // H          # 64
HH = H // 2          # 4 local heads per core
E = HH * Dh          # 256 local attention channels
EPS = 1e-10
N_CORES = 8
OWN = K // 2         # 1024 rows per core after the exchange

_CACHE = {}
DEBUG = False


def _build():
    nc = bacc.Bacc("TRN2", target_bir_lowering=False, debug=False,
                   num_devices=N_CORES)

    xt_d = nc.dram_tensor("xt", [D, K], BF16, kind="ExternalInput")
    xbo_d = nc.dram_tensor("xbo", [OWN, D], F32, kind="ExternalInput")
    wq_d = nc.dram_tensor("wq", [D, E], BF16, kind="ExternalInput")
    wk_d = nc.dram_tensor("wk", [D, E], BF16, kind="ExternalInput")
    wv_d = nc.dram_tensor("wv", [D, E], BF16, kind="ExternalInput")
    bq_d = nc.dram_tensor("bqc", [128, 2], F32, kind="ExternalInput")
    bk_d = nc.dram_tensor("bkc", [128, 2], F32, kind="ExternalInput")
    bv_d = nc.dram_tensor("bvr", [1, E], F32, kind="ExternalInput")
    wo_d = nc.dram_tensor("wo", [E, D], BF16, kind="ExternalInput")
    w1_d = nc.dram_tensor("w1", [D, F], BF16, kind="ExternalInput")
    b1_d = nc.dram_tensor("b1c", [128, 16], F32, kind="ExternalInput")
    w2_d = nc.dram_tensor("w2", [F, D], BF16, kind="ExternalInput")
    b2_d = nc.dram_tensor("b2r", [1, D], F32, kind="ExternalInput")
    g1_d = nc.dram_tensor("g1r", [1, D], F32, kind="ExternalInput")
    be1_d = nc.dram_tensor("be1r", [1, D], F32, kind="ExternalInput")
    g2_d = nc.dram_tensor("g2r", [1, D], F32, kind="ExternalInput")
    be2_d = nc.dram_tensor("be2r", [1, D], F32, kind="ExternalInput")
    out_d = nc.dram_tensor("out", [OWN, D], F32, kind="ExternalOutput")
    if DEBUG:
        dbg_kt = nc.dram_tensor("dbg_kt", [128, K], BF16,
                                kind="ExternalOutput")
        dbg_qt = nc.dram_tensor("dbg_qt", [128, K], BF16,
                                kind="ExternalOutput")
        dbg_va = nc.dram_tensor("dbg_va", [128, HH * 128], BF16,
                                kind="ExternalOutput")
        dbg_ac = nc.dram_tensor("dbg_ac", [2, 128, K], BF16,
                                kind="ExternalOutput")
        dbg_rs = nc.dram_tensor("dbg_rs", [OWN, D], BF16,
                                kind="ExternalOutput")
        dbg_h1 = nc.dram_tensor("dbg_h1", [128, D], F32,
                                kind="ExternalOutput")

    def bcast(dram, npart, n):
        return bass.AP(tensor=dram, offset=0, ap=[[0, npart], [1, n]])

    with tile.TileContext(nc) as tc:
        import contextlib
        stack = contextlib.ExitStack()
        with stack:
            singles = stack.enter_context(tc.tile_pool(name="singles", bufs=1))
            dram = stack.enter_context(
                tc.tile_pool(name="dram", bufs=1, space="DRAM"))
            drp = stack.enter_context(
                tc.tile_pool(name="drp", bufs=4, space="DRAM"))

            # ---- input DMAs in first-use order -------------------------
            pw = stack.enter_context(tc.tile_pool(name="pw", bufs=1))
            xT = [pw.tile([128, K], BF16, name=f"xT{i}") for i in range(4)]
            for kb in range(4):
                for dc in range(4):
                    nc.sync.dma_start(
                        out=xT[dc][:, kb * 512:(kb + 1) * 512],
                        in_=xt_d[dc * 128:(dc + 1) * 128,
                                 kb * 512:(kb + 1) * 512])
            wk_sb = [pw.tile([128, E], BF16, name=f"wk{i}") for i in range(4)]
            for dc in range(4):
                nc.sync.dma_start(out=wk_sb[dc],
                                  in_=wk_d[dc * 128:(dc + 1) * 128, :])
            wq_sb = [pw.tile([128, E], BF16, name=f"wq{i}") for i in range(4)]
            for dc in range(4):
                nc.sync.dma_start(out=wq_sb[dc],
                                  in_=wq_d[dc * 128:(dc + 1) * 128, :])
            wv_sb = [pw.tile([128, E], BF16, name=f"wv{i}") for i in range(4)]
            for dc in range(4):
                nc.sync.dma_start(out=wv_sb[dc],
                                  in_=wv_d[dc * 128:(dc + 1) * 128, :])
            bk_col = singles.tile([128, 2], F32)
            nc.sync.dma_start(out=bk_col, in_=bk_d[:, :])
            bq_col = singles.tile([128, 2], F32)
            nc.sync.dma_start(out=bq_col, in_=bq_d[:, :])
            b1_col = singles.tile([128, 16], F32)
            nc.sync.dma_start(out=b1_col, in_=b1_d[:, :])
            wo_sb = [pw.tile([128, D], BF16, name=f"wo{i}") for i in range(2)]
            for cc in range(2):
                nc.sync.dma_start(out=wo_sb[cc],
                                  in_=wo_d[cc * 128:(cc + 1) * 128, :])
            # late-use inputs last (needed only in phase 3)
            w1_sb = [pw.tile([128, F], BF16, name=f"w1_{i}") for i in range(4)]
            for dc in range(4):
                for fb in range(4):
                    nc.sync.dma_start(
                        out=w1_sb[dc][:, fb * 512:(fb + 1) * 512],
                        in_=w1_d[dc * 128:(dc + 1) * 128,
                                 fb * 512:(fb + 1) * 512])
            w2_sb = [pw.tile([128, D], BF16, name=f"w2_{i}") for i in range(16)]
            for fc in range(16):
                nc.sync.dma_start(out=w2_sb[fc],
                                  in_=w2_d[fc * 128:(fc + 1) * 128, :])
            xbo_sb = [pw.tile([128, D], F32, name=f"xbo{i}") for i in range(8)]
            for qt in range(8):
                nc.sync.dma_start(out=xbo_sb[qt],
                                  in_=xbo_d[qt * 128:(qt + 1) * 128, :])

            # broadcasts (gpsimd software DMA handles 0-stride partitions)
            bv_bc = singles.tile([128, E], F32)
            nc.gpsimd.dma_start(out=bv_bc, in_=bcast(bv_d, 128, E))
            g1_bc = singles.tile([128, D], F32)
            nc.gpsimd.dma_start(out=g1_bc, in_=bcast(g1_d, 128, D))
            beb2_bc = singles.tile([128, D], F32)
            nc.gpsimd.dma_start(out=beb2_bc, in_=bcast(be1_d, 128, D))
            g2_bc = singles.tile([128, D], F32)
            nc.gpsimd.dma_start(out=g2_bc, in_=bcast(g2_d, 128, D))
            be2_bc = singles.tile([128, D], F32)
            nc.gpsimd.dma_start(out=be2_bc, in_=bcast(be2_d, 128, D))

            # ---- static tiles ------------------------------------------
            ident = singles.tile([128, 128], F32)
            masks.make_identity(nc, ident[:])
            tri01 = singles.tile([128, 128], BF16)
            nc.gpsimd.memset(tri01, 1.0)
            # keep 1.0 where q - k >= 0 (partition = key, free = query)
            nc.gpsimd.affine_select(
                out=tri01, in_=tri01, compare_op=ALU.is_ge,
                fill=0.0, base=0, pattern=[[1, 128]], channel_multiplier=-1)
            eps_t = singles.tile([128, 1], F32)
            nc.vector.memset(eps_t, EPS)

            # ---- persistent activation tensors -------------------------
            kt_pool = stack.enter_context(tc.tile_pool(name="kt", bufs=1))
            qt_pool = stack.enter_context(tc.tile_pool(name="qt", bufs=1))
            va_pool = stack.enter_context(tc.tile_pool(name="va", bufs=1))
            ac_pool = stack.enter_context(tc.tile_pool(name="ac", bufs=1))
            kT = [kt_pool.tile([128, K], BF16, name=f"kT{i}") for i in range(2)]
            qT = [qt_pool.tile([128, K], BF16, name=f"qT{i}") for i in range(2)]
            va = [va_pool.tile([128, HH, 128], BF16, name=f"va{i}")
                  for i in range(K // 128)]
            ac = [ac_pool.tile([128, K], BF16, name=f"ac{i}") for i in range(2)]
            for kt_i in range(K // 128):
                nc.gpsimd.memset(va[kt_i][:, :, Dh:128], 0.0)
                nc.gpsimd.memset(va[kt_i][:, :, Dh:Dh + 1], 1.0)

            h1_pool = stack.enter_context(tc.tile_pool(name="h1", bufs=1))
            h1 = [h1_pool.tile([128, D], F32, name=f"h1_{i}") for i in range(8)]
            h1t_pool = stack.enter_context(tc.tile_pool(name="h1t", bufs=1))
            h1T = h1t_pool.tile([128, 4 * OWN], BF16, name="h1T")
            lnp = stack.enter_context(tc.tile_pool(name="lnp", bufs=4))

            # ---------------- phase 1: projections ----------------------
            with tc.tile_pool(name="ps_p", bufs=4, space="PSUM") as ps_p:
                for cc in range(2):
                    for kb in range(4):
                        pp = ps_p.tile([128, 512], F32, name="pp")
                        for dc in range(4):
                            nc.tensor.matmul(
                                pp[:],
                                wk_sb[dc][:, cc * 128:(cc + 1) * 128],
                                xT[dc][:, kb * 512:(kb + 1) * 512],
                                start=(dc == 0), stop=(dc == 3))
                        nc.vector.tensor_scalar(
                            out=kT[cc][:, kb * 512:(kb + 1) * 512],
                            in0=pp[:], scalar1=bk_col[:, cc:cc + 1],
                            scalar2=None, op0=ALU.add)
                for cc in range(2):
                    for kb in range(4):
                        pp = ps_p.tile([128, 512], F32, name="pp")
                        for dc in range(4):
                            nc.tensor.matmul(
                                pp[:],
                                wq_sb[dc][:, cc * 128:(cc + 1) * 128],
                                xT[dc][:, kb * 512:(kb + 1) * 512],
                                start=(dc == 0), stop=(dc == 3))
                        nc.vector.tensor_scalar(
                            out=qT[cc][:, kb * 512:(kb + 1) * 512],
                            in0=pp[:], scalar1=bq_col[:, cc:cc + 1],
                            scalar2=None, op0=ALU.add)
                # v rows (4 local heads at once); ones column pre-memset
                for kt_i in range(K // 128):
                    vp = ps_p.tile([128, E], F32, name="vp")
                    for dc in range(4):
                        nc.tensor.matmul(
                            vp[:],
                            xT[dc][:, kt_i * 128:(kt_i + 1) * 128],
                            wv_sb[dc][:], start=(dc == 0), stop=(dc == 3))
                    nc.vector.tensor_add(
                        out=va[kt_i][:, :, 0:Dh],
                        in0=vp[:].rearrange("p (h e) -> p h e", h=HH),
                        in1=bv_bc[:].rearrange("p (h e) -> p h e", h=HH))

            # ---------------- phase 2: attention + Wo partials + RS -----
            rsA_in = dram.tile([1024, D], BF16, name="rsA_in")
            rsB_in = dram.tile([1024, D], BF16, name="rsB_in")
            rsA_out = dram.tile([512, D], BF16, name="rsA_out")
            rsB_out = dram.tile([512, D], BF16, name="rsB_out")

            # diag packing: jp0 tile holds m0 (w=512) @ col 0 and
            # m1 (w=384) @ col 512; jp1 tile holds m2 (w=256) @ col 0 and
            # m3 (w=128) @ col 256.
            DIAG = [  # (m, colbase, width)
                [(0, 0, 512), (1, 512, 384)],
                [(2, 0, 256), (3, 256, 128)],
            ]

            def attn_pair(cc, qb, ps_s, ps_att, expp, bcp):
                """Causal attention for heads (2cc, 2cc+1), query block qb."""
                qs = qb * 512
                att = ps_att.tile([128, 1024], F32, name="att")
                n_av = [0, 0]

                def av(hh, kb, lo, exp_ap, stop):
                    # two interleaved accumulation groups (hh=0 cols 0:512,
                    # hh=1 cols 512:1024) share the att tile
                    nc.tensor.matmul(
                        att[:, hh * 512 + lo:hh * 512 + 512],
                        va[kb][:, 2 * cc + hh, :], exp_ap,
                        start=(n_av[hh] == 0), stop=stop,
                        skip_group_check=True)
                    n_av[hh] += 1

                # full key blocks: one [128,1024] tile = 1 kb x 2 heads
                for kb in range(4 * qb):
                    s2 = ps_s.tile([128, 1024], F32, name="s2")
                    for hh in range(2):
                        nc.tensor.matmul(
                            s2[:, hh * 512:(hh + 1) * 512],
                            kT[cc][hh * 64:(hh + 1) * 64,
                                   kb * 128:(kb + 1) * 128],
                            qT[cc][hh * 64:(hh + 1) * 64, qs:qs + 512],
                            start=True, stop=True)
                    expT = expp.tile([128, 1024], BF16, name="expT")
                    nc.scalar.activation(out=expT[:], in_=s2[:],
                                         func=AF.Exp, scale=0.125)
                    for hh in range(2):
                        av(hh, kb, 0, expT[:, hh * 512:(hh + 1) * 512],
                           stop=False)
                # diagonal: per jp, one tile per head half (no stale cols)
                for jp in range(2):
                    dt = [ps_s.tile([128, 1024], F32, name="s2")
                          for _ in range(2)]
                    for (m, cb, w) in DIAG[jp]:
                        kb = 4 * qb + m
                        lo = m * 128
                        for hh in range(2):
                            nc.tensor.matmul(
                                dt[hh][:, cb:cb + w],
                                kT[cc][hh * 64:(hh + 1) * 64,
                                       kb * 128:(kb + 1) * 128],
                                qT[cc][hh * 64:(hh + 1) * 64,
                                       qs + lo:qs + 512],
                                start=True, stop=True)
                    tot = sum(w for (_, _, w) in DIAG[jp])
                    expd = [expp.tile([128, 1024], BF16, name="expT")
                            for _ in range(2)]
                    for hh in range(2):
                        nc.scalar.activation(out=expd[hh][:, 0:tot],
                                             in_=dt[hh][:, 0:tot],
                                             func=AF.Exp, scale=0.125)
                    # zero the still-masked triangle (k > q) on DVE
                    for hh in range(2):
                        for (m, cb, w) in DIAG[jp]:
                            nc.vector.tensor_mul(
                                out=expd[hh][:, cb:cb + 128],
                                in0=expd[hh][:, cb:cb + 128], in1=tri01[:])
                    for (m, cb, w) in DIAG[jp]:
                        kb = 4 * qb + m
                        lo = m * 128
                        last = (jp == 1 and m == 3)
                        for hh in range(2):
                            av(hh, kb, lo, expd[hh][:, cb:cb + w], stop=last)
                # normalize: drain att to SBUF first so its PSUM slot frees
                # immediately -- the DRAM round-trip broadcast below can
                # stall >10us when a ReduceScatter hogs the DMA queues,
                # which otherwise blocks the next group's AV accumulation.
                # (reciprocal_approx_fast only works at base partition 0, so
                # the den row is copied down to partition 0 from SBUF.)
                araw = bcp.tile([128, 1024], F32, name="araw")
                nc.vector.tensor_copy(out=araw[:], in_=att[:])
                den = bcp.tile([1, 1024], F32, name="den")
                nc.vector.tensor_copy(out=den[:], in_=araw[64:65, :])
                rec = bcp.tile([1, 1024], F32, name="rec")
                nc.vector.reciprocal_approx_fast(out=rec[:], in_=den[:])
                rec_dr = drp.tile([1, 1024], F32, name="rec_dr")
                nc.sync.dma_start(out=rec_dr[:], in_=rec[:])
                bc_sb = bcp.tile([64, 1024], F32, name="bc_sb")
                nc.sync.dma_start(out=bc_sb[:], in_=bass.AP(
                    tensor=rec_dr[:].tensor, offset=rec_dr[:].offset,
                    ap=[[0, 64], [1, 1024]]))
                for hh in range(2):
                    nc.vector.tensor_mul(
                        out=ac[cc][hh * 64:(hh + 1) * 64, qs:qs + 512],
                        in0=araw[0:64, hh * 512:(hh + 1) * 512],
                        in1=bc_sb[:, hh * 512:(hh + 1) * 512])

            def wo_partial(qb, dest, row0, ps_s, wop, engs=None):
                """o_part[qb] = ac[:, qb]^T @ wo -> bf16 -> rs_in rows.
                engs: per-DMA trigger queues (default gpsimd). The last
                wo feeds RS B directly; spreading its rs_in writes across
                the then-idle sync/scalar rings gets the collective off
                ~6-10us earlier."""
                if engs is None:
                    engs = [nc.gpsimd] * 4
                for half in range(2):
                    o_ps = ps_s.tile([128, 1024], F32, name="s2")
                    for g in range(2):
                        qt2 = half * 2 + g
                        for cc in range(2):
                            nc.tensor.matmul(
                                o_ps[:, g * 512:(g + 1) * 512],
                                ac[cc][:, qb * 512 + qt2 * 128:
                                       qb * 512 + (qt2 + 1) * 128],
                                wo_sb[cc][:], start=(cc == 0), stop=(cc == 1))
                    o_sb = wop.tile([128, 1024], BF16, name="o_sb")
                    nc.vector.tensor_copy(out=o_sb[:], in_=o_ps[:])
                    for g in range(2):
                        qt2 = half * 2 + g
                        engs[half * 2 + g].dma_start(
                            out=dest[row0 + qt2 * 128:row0 + (qt2 + 1) * 128,
                                     :],
                            in_=o_sb[:, g * 512:(g + 1) * 512])

            RG = [[0, 1], [2, 3], [4, 5], [6, 7]]
            # qb order (0,2,1,3); RS A = [qb0|qb2] launches mid-attention,
            # RS B = [qb1|qb3] at the end.
            wo_args = {0: (rsA_in, 0), 2: (rsA_in, 512),
                       1: (rsB_in, 0), 3: (rsB_in, 512)}
            with tc.tile_pool(name="ps_s", bufs=2, space="PSUM") as ps_s, \
                 tc.tile_pool(name="ps_att", bufs=2, space="PSUM") as ps_att, \
                 tc.tile_pool(name="expp", bufs=3) as expp, \
                 tc.tile_pool(name="bcp", bufs=2) as bcp, \
                 tc.tile_pool(name="wop", bufs=3) as wop:
                order = [0, 2, 1, 3]
                for qi, qb in enumerate(order):
                    attn_pair(0, qb, ps_s, ps_att, expp, bcp)
                    if qi >= 1:
                        pqb = order[qi - 1]
                        dest, row0 = wo_args[pqb]
                        wo_partial(pqb, dest, row0, ps_s, wop)
                    attn_pair(1, qb, ps_s, ps_att, expp, bcp)
                    if qi == 2:
                        nc.gpsimd.collective_compute(
                            "ReduceScatter", ALU.add, replica_groups=RG,
                            ins=[rsA_in[:]], outs=[rsA_out[:]])
                wo_partial(3, rsB_in, 512, ps_s, wop,
                           engs=[nc.scalar, nc.sync, nc.scalar, nc.sync])
                nc.gpsimd.collective_compute(
                    "ReduceScatter", ALU.add, replica_groups=RG,
                    ins=[rsB_in[:]], outs=[rsB_out[:]])
                if DEBUG:
                    nc.sync.dma_start(out=dbg_kt[:, :], in_=kT[0][:])
                    nc.sync.dma_start(out=dbg_qt[:, :], in_=qT[0][:])
                    nc.sync.dma_start(out=dbg_va[:, :],
                                      in_=va[0][:].rearrange(
                                          "p h e -> p (h e)"))
                    for cc in range(2):
                        nc.sync.dma_start(out=dbg_ac[cc, :, :], in_=ac[cc][:])
                    nc.sync.dma_start(out=dbg_rs[0:512, :], in_=rsA_out[:])
                    nc.sync.dma_start(out=dbg_rs[512:1024, :],
                                      in_=rsB_out[:])

            # ---------------- phase 3: Wo + LN1 + FFN + LN2 -------------
            def layer_norm_core(pre, dst):
                stats = lnp.tile([128, 6], F32, name="ln_stats")
                nc.vector.bn_stats(out=stats[:], in_=pre[:])
                mv = lnp.tile([128, 2], F32, name="ln_mv")
                nc.vector.bn_aggr(out=mv[:], in_=stats[:])
                rstd = lnp.tile([128, 1], F32, name="ln_rstd")
                nc.scalar.activation(out=rstd[:], in_=mv[:, 1:2],
                                     func=AF.Sqrt, bias=eps_t[:])
                nc.vector.reciprocal(out=rstd[:], in_=rstd[:])
                nc.vector.tensor_scalar(
                    out=dst, in0=pre[:], scalar1=mv[:, 0:1],
                    scalar2=rstd[:], op0=ALU.subtract, op1=ALU.mult)

            def wo_ln1(half, src, orp):
                """rs_out read + residual + LN1 for rows of this half."""
                for qt2 in range(4):
                    qt = half * 4 + qt2
                    o_rs = orp.tile([128, D], BF16, name="o_rs")
                    nc.sync.dma_start(
                        out=o_rs,
                        in_=src[qt2 * 128:(qt2 + 1) * 128, :])
                    pre = lnp.tile([128, D], F32, name="ln_pre")
                    nc.vector.tensor_add(out=pre[:], in0=o_rs[:],
                                         in1=xbo_sb[qt][:])
                    layer_norm_core(pre, h1[qt][:])

            def transp(half, ps_aux):
                """transpose n1 into h1T, then fold g1/beb2 into h1."""
                for qt2 in range(4):
                    qt = half * 4 + qt2
                    trp = ps_aux.tile([128, D], F32, name="aux")
                    for dc in range(4):
                        nc.tensor.transpose(
                            trp[:, dc * 128:(dc + 1) * 128],
                            h1[qt][:, dc * 128:(dc + 1) * 128], ident[:])
                    nc.vector.tensor_copy(
                        out=h1T[:].rearrange("p (dc c) -> p dc c", dc=4)
                            [:, :, qt * 128:(qt + 1) * 128],
                        in_=trp[:].rearrange("p (dc c) -> p dc c", dc=4))
                for qt2 in range(4):
                    qt = half * 4 + qt2
                    nc.gpsimd.tensor_mul(out=h1[qt][:], in0=h1[qt][:],
                                         in1=g1_bc[:])
                    nc.gpsimd.tensor_add(out=h1[qt][:], in0=h1[qt][:],
                                         in1=beb2_bc[:])

            def ffn1(half, ps_f1, fap, fa_out):
                for fc in range(16):
                    fp_ps = ps_f1.tile([128, 512], F32, name="fp_ps")
                    for dc in range(4):
                        nc.tensor.matmul(
                            fp_ps[:],
                            w1_sb[dc][:, fc * 128:(fc + 1) * 128],
                            h1T[:, dc * OWN + half * 512:
                                dc * OWN + (half + 1) * 512],
                            start=(dc == 0), stop=(dc == 3))
                    fa_t = fap.tile([128, 512], BF16, name=f"fa{fc}")
                    nc.scalar.activation(out=fa_t[:], in_=fp_ps[:],
                                         func=AF.Relu,
                                         bias=b1_col[:, fc:fc + 1])
                    fa_out.append(fa_t)

            def ffn2(half, ps_f2, fa, outp):
                for qt2 in range(4):
                    qt = half * 4 + qt2
                    ff2_ps = ps_f2.tile([128, D], F32, name="ff2")
                    for fc in range(16):
                        nc.tensor.matmul(
                            ff2_ps[:],
                            fa[fc][:, qt2 * 128:(qt2 + 1) * 128],
                            w2_sb[fc][:], start=(fc == 0), stop=(fc == 15))
                    # h1 already holds n1*g1 + (be1+b2)
                    pre = lnp.tile([128, D], F32, name="ln_pre")
                    nc.vector.tensor_add(out=pre[:], in0=ff2_ps[:],
                                         in1=h1[qt][:])
                    out_sb = outp.tile([128, D], F32, name="out_sb")
                    layer_norm_core(pre, out_sb[:])
                    nc.gpsimd.tensor_mul(out=out_sb[:], in0=out_sb[:],
                                         in1=g2_bc[:])
                    nc.gpsimd.tensor_add(out=out_sb[:], in0=out_sb[:],
                                         in1=be2_bc[:])
                    nc.sync.dma_start(
                        out=out_d[qt * 128:(qt + 1) * 128, :], in_=out_sb[:])

            with tc.tile_pool(name="ps_aux", bufs=2, space="PSUM") as ps_aux, \
                 tc.tile_pool(name="ps_f1", bufs=3, space="PSUM") as ps_f1, \
                 tc.tile_pool(name="ps_f2", bufs=2, space="PSUM") as ps_f2, \
                 tc.tile_pool(name="fap", bufs=1) as fap, \
                 tc.tile_pool(name="orp", bufs=3) as orp, \
                 tc.tile_pool(name="outp", bufs=2) as outp:
                fa0, fa1 = [], []
                wo_ln1(0, rsA_out, orp)
                if DEBUG:
                    nc.sync.dma_start(out=dbg_h1[:, :], in_=h1[0][:])
                transp(0, ps_aux)
                ffn1(0, ps_f1, fap, fa0)
                # half-1 LN chain emitted before half-0's LN2 so the DVE
                # order matches readiness (RS B lands before ffn2(0) ends).
                wo_ln1(1, rsB_out, orp)
                ffn2(0, ps_f2, fa0, outp)
                transp(1, ps_aux)
                ffn1(1, ps_f1, fap, fa1)
                ffn2(1, ps_f2, fa1, outp)

    nc.compile()
    return nc


def _get_nc():
    if "nc" not in _CACHE:
        _CACHE["nc"] = _build()
    return _CACHE["nc"]


def _make_in_maps(x, Wq, bq, Wk, bk, Wv, bv, Wo, bo, W1, b1, W2, b2, g1, be1,
                  g2, be2):
    bf = ml_dtypes.bfloat16
    x = np.ascontiguousarray(np.asarray(x, dtype=np.float32))
    Wq, Wk, Wv = (np.asarray(w, np.float32) for w in (Wq, Wk, Wv))
    bo = np.asarray(bo, np.float32)
    g1f = np.asarray(g1, np.float32)
    be1f = np.asarray(be1, np.float32)
    w1f = np.asarray(W1, np.float32)
    w1b = np.ascontiguousarray((g1f[:, None] * w1f).astype(bf))
    b1f = np.asarray(b1, np.float32) + be1f @ w1f
    w2b = np.ascontiguousarray(np.asarray(W2, np.float32).astype(bf))
    wof = np.asarray(Wo, np.float32)
    b1c = np.ascontiguousarray(b1f.reshape(16, 128).T)
    in_maps = []
    for c in range(N_CORES):
        n, s = divmod(c, 2)
        hsel = slice(HH * s, HH * s + HH)
        in_maps.append({
            "xt": np.ascontiguousarray(x[n].T.astype(bf)),
            "xbo": np.ascontiguousarray(x[n, OWN * s:OWN * s + OWN] + bo),
            "wq": np.ascontiguousarray(
                Wq[hsel].transpose(1, 0, 2).reshape(D, E).astype(bf)),
            "wk": np.ascontiguousarray(
                Wk[hsel].transpose(1, 0, 2).reshape(D, E).astype(bf)),
            "wv": np.ascontiguousarray(
                Wv[hsel].transpose(1, 0, 2).reshape(D, E).astype(bf)),
            "bqc": np.ascontiguousarray(
                np.asarray(bq, np.float32)[hsel].reshape(2, 128).T),
            "bkc": np.ascontiguousarray(
                np.asarray(bk, np.float32)[hsel].reshape(2, 128).T),
            "bvr": np.ascontiguousarray(
                np.asarray(bv, np.float32)[hsel]).reshape(1, E),
            "wo": np.ascontiguousarray(wof[E * s:E * s + E].astype(bf)),
            "w1": w1b,
            "b1c": b1c,
            "w2": w2b,
            "b2r": np.asarray(b2, np.float32).reshape(1, D),
            # beb2 rides in the be1r slot: residual bias be1 + b2
            "g1r": np.asarray(g1, np.float32).reshape(1, D),
            "be1r": (be1f + np.asarray(b2, np.float32)).reshape(1, D),
            "g2r": np.asarray(g2, np.float32).reshape(1, D),
            "be2r": np.asarray(be2, np.float32).reshape(1, D),
        })
    return in_maps


def kernel(x, Wq, bq, Wk, bk, Wv, bv, Wo, bo, W1, b1, W2, b2, g1, be1, g2,
           be2, mask=None, **_unused):
    nc = _get_nc()
    in_maps = _make_in_maps(x, Wq, bq, Wk, bk, Wv, bv, Wo, bo, W1, b1, W2, b2,
                            g1, be1, g2, be2)
    res = bass_utils.run_bass_kernel_spmd(
        nc, in_maps, core_ids=list(range(N_CORES)))
    y = np.empty((N, K, D), np.float32)
    for c in range(N_CORES):
        n, s = divmod(c, 2)
        y[n, OWN * s:OWN * s + OWN] = res.results[c]["out"]
    return y


def kernel_timed(x, Wq, bq, Wk, bk, Wv, bv, Wo, bo, W1, b1, W2, b2, g1, be1,
                 g2, be2, mask=None, trace_cores=None, **_unused):
    """Run with NTFF tracing; returns BassKernelResults (exec_time_ns etc)."""
    nc = _get_nc()
    in_maps = _make_in_maps(x, Wq, bq, Wk, bk, Wv, bv, Wo, bo, W1, b1, W2, b2,
                            g1, be1, g2, be2)
    if trace_cores is None:
        trace_cores = list(range(N_CORES))
    return bass_utils.run_bass_kernel_spmd(
        nc, in_maps, core_ids=list(range(N_CORES)), trace=True,
        trace_cores=trace_cores)


# revision 37
# speedup vs baseline: 1.0318x; 1.0251x over previous
"""Decoder block (8-head causal attention + FFN + 2x layernorm) on 8 trn2 cores.

Problem: x (4, 2048, 512) fp32; per-head Wq/Wk/Wv (8, 512, 64); Wo (512, 512);
FFN 512->2048->512; causal mask; two post-residual layernorms.

Sharding (uniform SPMD program, 8 cores): core c -> (batch n = c//2,
head-half s = c%2). Each core computes Q/K/V for its 4 heads over the full
2048-token sequence of its batch and causal attention for all 2048 queries.
Each core computes its Wo partial (contraction over its 256 channels) for
all rows in bf16; two pairwise ReduceScatters sum the partials and hand each
core its own 1024 rows. Each core then runs residual+LN1, FFN and
residual+LN2 for its rows. Host reassembles.

v2 restructure vs v1 baseline (281us):
 - input DMAs ordered by first use (xT/wk/wq/wv first, w1/w2/xbo last) so
   the first matmul starts ~8us in instead of ~30us.
 - scores packed 2 heads per step via row-tiled concurrent matmuls (K=64,
   tile rows 0-63 = even head, 64-127 = odd head) -> ~2x score PE time.
 - qT is one tile per channel-chunk (even head rows 0:64, odd 64:128) --
   the natural PSUM layout; halves the q drain calls and SBUF.
 - diagonal blocks packed 2-per-psum-tile per head half: fewer, larger exp
   calls with no stale columns.
 - ACT does exp only (plus phase-3 relu/sqrt after all exps -> 2 table
   loads); k/q/v drains on DVE; triangle masks on DVE (127ns vs 406 gpsimd);
   LN gain/bias on gpsimd; denominator reciprocal reads PSUM directly
   ([1,1024] covers both heads, one DRAM round-trip broadcast per group).
 - Wo partial PSUM shares the score pool slots (PSUM = 2x[128,1024] scores
   + 2x[128,1024] att accumulators = exactly 8 banks).
 - phase 3 ordered so the half-1 LN chain is emitted before half-0's LN2
   (no DVE FIFO head-block while RS B is in flight).

All matmuls bf16 with fp32 PSUM accumulation; softmax without
max-subtraction; denominator via ones-column in V.
"""

import sys

sys.path.insert(0, "/opt/trn_rl_repo")

import numpy as np
import ml_dtypes

import concourse.bacc as bacc
import concourse.bass as bass
import concourse.mybir as mybir
import concourse.tile as tile
from concourse import bass_utils, masks

F32 = mybir.dt.float32
BF16 = mybir.dt.bfloat16
AF = mybir.ActivationFunctionType
ALU = mybir.AluOpType

N, K, D, H, F = 4, 2048, 512, 8, 2048
Dh = D // H          # 64
HH = H // 2          # 4 local heads per core
E = HH * Dh          # 256 local attention channels
EPS = 1e-10
N_CORES = 8
OWN = K // 2         # 1024 rows per core after the exchange

_CACHE = {}
DEBUG = False


def _build():
    nc = bacc.Bacc("TRN2", target_bir_lowering=False, debug=False,
                   num_devices=N_CORES)

    xt_d = nc.dram_tensor("xt", [D, K], BF16, kind="ExternalInput")
    xbo_d = nc.dram_tensor("xbo", [OWN, D], F32, kind="ExternalInput")
    wq_d = nc.dram_tensor("wq", [D, E], BF16, kind="ExternalInput")
    wk_d = nc.dram_tensor("wk", [D, E], BF16, kind="ExternalInput")
    wv_d = nc.dram_tensor("wv", [D, E], BF16, kind="ExternalInput")
    bq_d = nc.dram_tensor("bqc", [128, 2], F32, kind="ExternalInput")
    bk_d = nc.dram_tensor("bkc", [128, 2], F32, kind="ExternalInput")
    bv_d = nc.dram_tensor("bvr", [1, E], F32, kind="ExternalInput")
    wo_d = nc.dram_tensor("wo", [E, D], BF16, kind="ExternalInput")
    w1_d = nc.dram_tensor("w1", [D, F], BF16, kind="ExternalInput")
    b1_d = nc.dram_tensor("b1c", [128, 16], F32, kind="ExternalInput")
    w2_d = nc.dram_tensor("w2", [F, D], BF16, kind="ExternalInput")
    b2_d = nc.dram_tensor("b2r", [1, D], F32, kind="ExternalInput")
    g1_d = nc.dram_tensor("g1r", [1, D], F32, kind="ExternalInput")
    be1_d = nc.dram_tensor("be1r", [1, D], F32, kind="ExternalInput")
    g2_d = nc.dram_tensor("g2r", [1, D], F32, kind="ExternalInput")
    be2_d = nc.dram_tensor("be2r", [1, D], F32, kind="ExternalInput")
    out_d = nc.dram_tensor("out", [OWN, D], F32, kind="ExternalOutput")
    if DEBUG:
        dbg_kt = nc.dram_tensor("dbg_kt", [128, K], BF16,
                                kind="ExternalOutput")
        dbg_qt = nc.dram_tensor("dbg_qt", [128, K], BF16,
                                kind="ExternalOutput")
        dbg_va = nc.dram_tensor("dbg_va", [128, HH * 128], BF16,
                                kind="ExternalOutput")
        dbg_ac = nc.dram_tensor("dbg_ac", [2, 128, K], BF16,
                                kind="ExternalOutput")
        dbg_rs = nc.dram_tensor("dbg_rs", [OWN, D], BF16,
                                kind="ExternalOutput")
        dbg_h1 = nc.dram_tensor("dbg_h1", [128, D], F32,
                                kind="ExternalOutput")

    def bcast(dram, npart, n):
        return bass.AP(tensor=dram, offset=0, ap=[[0, npart], [1, n]])

    with tile.TileContext(nc) as tc:
        import contextlib
        stack = contextlib.ExitStack()
        with stack:
            singles = stack.enter_context(tc.tile_pool(name="singles", bufs=1))
            dram = stack.enter_context(
                tc.tile_pool(name="dram", bufs=1, space="DRAM"))
            drp = stack.enter_context(
                tc.tile_pool(name="drp", bufs=4, space="DRAM"))

            # ---- input DMAs in first-use order -------------------------
            pw = stack.enter_context(tc.tile_pool(name="pw", bufs=1))
            xT = [pw.tile([128, K], BF16, name=f"xT{i}") for i in range(4)]
            for kb in range(4):
                for dc in range(4):
                    nc.sync.dma_start(
                        out=xT[dc][:, kb * 512:(kb + 1) * 512],
                        in_=xt_d[dc * 128:(dc + 1) * 128,
                                 kb * 512:(kb + 1) * 512])
            wk_sb = [pw.tile([128, E], BF16, name=f"wk{i}") for i in range(4)]
            for dc in range(4):
                nc.sync.dma_start(out=wk_sb[dc],
                                  in_=wk_d[dc * 128:(dc + 1) * 128, :])
            wq_sb = [pw.tile([128, E], BF16, name=f"wq{i}") for i in range(4)]
            for dc in range(4):
                nc.sync.dma_start(out=wq_sb[dc],
                                  in_=wq_d[dc * 128:(dc + 1) * 128, :])
            wv_sb = [pw.tile([128, E], BF16, name=f"wv{i}") for i in range(4)]
            for dc in range(4):
                nc.sync.dma_start(out=wv_sb[dc],
                                  in_=wv_d[dc * 128:(dc + 1) * 128, :])
            bk_col = singles.tile([128, 2], F32)
            nc.sync.dma_start(out=bk_col, in_=bk_d[:, :])
            bq_col = singles.tile([128, 2], F32)
            nc.sync.dma_start(out=bq_col, in_=bq_d[:, :])
            b1_col = singles.tile([128, 16], F32)
            nc.sync.dma_start(out=b1_col, in_=b1_d[:, :])
            wo_sb = [pw.tile([128, D], BF16, name=f"wo{i}") for i in range(2)]
            for cc in range(2):
                nc.sync.dma_start(out=wo_sb[cc],
                                  in_=wo_d[cc * 128:(cc + 1) * 128, :])
            # late-use inputs last (needed only in phase 3)
            w1_sb = [pw.tile([128, F], BF16, name=f"w1_{i}") for i in range(4)]
            for dc in range(4):
                for fb in range(4):
                    nc.sync.dma_start(
                        out=w1_sb[dc][:, fb * 512:(fb + 1) * 512],
                        in_=w1_d[dc * 128:(dc + 1) * 128,
                                 fb * 512:(fb + 1) * 512])
            w2_sb = [pw.tile([128, D], BF16, name=f"w2_{i}") for i in range(16)]
            for fc in range(16):
                nc.sync.dma_start(out=w2_sb[fc],
                                  in_=w2_d[fc * 128:(fc + 1) * 128, :])
            xbo_sb = [pw.tile([128, D], F32, name=f"xbo{i}") for i in range(8)]
            for qt in range(8):
                nc.sync.dma_start(out=xbo_sb[qt],
                                  in_=xbo_d[qt * 128:(qt + 1) * 128, :])

            # broadcasts (gpsimd software DMA handles 0-stride partitions)
            bv_bc = singles.tile([128, E], F32)
            nc.gpsimd.dma_start(out=bv_bc, in_=bcast(bv_d, 128, E))
            g1_bc = singles.tile([128, D], F32)
            nc.gpsimd.dma_start(out=g1_bc, in_=bcast(g1_d, 128, D))
            beb2_bc = singles.tile([128, D], F32)
            nc.gpsimd.dma_start(out=beb2_bc, in_=bcast(be1_d, 128, D))
            g2_bc = singles.tile([128, D], F32)
            nc.gpsimd.dma_start(out=g2_bc, in_=bcast(g2_d, 128, D))
            be2_bc = singles.tile([128, D], F32)
            nc.gpsimd.dma_start(out=be2_bc, in_=bcast(be2_d, 128, D))

            # ---- static tiles ------------------------------------------
            ident = singles.tile([128, 128], F32)
            masks.make_identity(nc, ident[:])
            tri01 = singles.tile([128, 128], BF16)
            nc.gpsimd.memset(tri01, 1.0)
            # keep 1.0 where q - k >= 0 (partition = key, free = query)
            nc.gpsimd.affine_select(
                out=tri01, in_=tri01, compare_op=ALU.is_ge,
                fill=0.0, base=0, pattern=[[1, 128]], channel_multiplier=-1)
            eps_t = singles.tile([128, 1], F32)
            nc.vector.memset(eps_t, EPS)

            # ---- persistent activation tensors -------------------------
            kt_pool = stack.enter_context(tc.tile_pool(name="kt", bufs=1))
            qt_pool = stack.enter_context(tc.tile_pool(name="qt", bufs=1))
            va_pool = stack.enter_context(tc.tile_pool(name="va", bufs=1))
            ac_pool = stack.enter_context(tc.tile_pool(name="ac", bufs=1))
            kT = [kt_pool.tile([128, K], BF16, name=f"kT{i}") for i in range(2)]
            qT = [qt_pool.tile([128, K], BF16, name=f"qT{i}") for i in range(2)]
            va = [va_pool.tile([128, HH, 128], BF16, name=f"va{i}")
                  for i in range(K // 128)]
            ac = [ac_pool.tile([128, K], BF16, name=f"ac{i}") for i in range(2)]
            for kt_i in range(K // 128):
                nc.gpsimd.memset(va[kt_i][:, :, Dh:128], 0.0)
                nc.gpsimd.memset(va[kt_i][:, :, Dh:Dh + 1], 1.0)

            h1_pool = stack.enter_context(tc.tile_pool(name="h1", bufs=1))
            h1 = [h1_pool.tile([128, D], F32, name=f"h1_{i}") for i in range(8)]
            h1t_pool = stack.enter_context(tc.tile_pool(name="h1t", bufs=1))
            h1T = h1t_pool.tile([128, 4 * OWN], BF16, name="h1T")
            lnp = stack.enter_context(tc.tile_pool(name="lnp", bufs=4))

            # ---------------- phase 1: projections ----------------------
            with tc.tile_pool(name="ps_p", bufs=4, space="PSUM") as ps_p:
                for cc in range(2):
                    for kb in range(4):
                        pp = ps_p.tile([128, 512], F32, name="pp")
                        for dc in range(4):
                            nc.tensor.matmul(
                                pp[:],
                                wk_sb[dc][:, cc * 128:(cc + 1) * 128],
                                xT[dc][:, kb * 512:(kb + 1) * 512],
                                start=(dc == 0), stop=(dc == 3))
                        nc.vector.tensor_scalar(
                            out=kT[cc][:, kb * 512:(kb + 1) * 512],
                            in0=pp[:], scalar1=bk_col[:, cc:cc + 1],
                            scalar2=None, op0=ALU.add)
                for cc in range(2):
                    for kb in range(4):
                        pp = ps_p.tile([128, 512], F32, name="pp")
                        for dc in range(4):
                            nc.tensor.matmul(
                                pp[:],
                                wq_sb[dc][:, cc * 128:(cc + 1) * 128],
                                xT[dc][:, kb * 512:(kb + 1) * 512],
                                start=(dc == 0), stop=(dc == 3))
                        nc.vector.tensor_scalar(
                            out=qT[cc][:, kb * 512:(kb + 1) * 512],
                            in0=pp[:], scalar1=bq_col[:, cc:cc + 1],
                            scalar2=None, op0=ALU.add)
                # v rows (4 local heads at once); ones column pre-memset
                for kt_i in range(K // 128):
                    vp = ps_p.tile([128, E], F32, name="vp")
                    for dc in range(4):
                        nc.tensor.matmul(
                            vp[:],
                            xT[dc][:, kt_i * 128:(kt_i + 1) * 128],
                            wv_sb[dc][:], start=(dc == 0), stop=(dc == 3))
                    nc.vector.tensor_add(
                        out=va[kt_i][:, :, 0:Dh],
                        in0=vp[:].rearrange("p (h e) -> p h e", h=HH),
                        in1=bv_bc[:].rearrange("p (h e) -> p h e", h=HH))

            # ---------------- phase 2: attention + Wo partials + RS -----
            rsA_in = dram.tile([1024, D], BF16, name="rsA_in")
            rsB_in = dram.tile([1024, D], BF16, name="rsB_in")
            rsA_out = dram.tile([512, D], BF16, name="rsA_out")
            rsB_out = dram.tile([512, D], BF16, name="rsB_out")

            # diag packing: jp0 tile holds m0 (w=512) @ col 0 and
            # m1 (w=384) @ col 512; jp1 tile holds m2 (w=256) @ col 0 and
            # m3 (w=128) @ col 256.
            DIAG = [  # (m, colbase, width)
                [(0, 0, 512), (1, 512, 384)],
                [(2, 0, 256), (3, 256, 128)],
            ]

            def attn_pair(cc, qb, ps_s, ps_att, expp, bcp):
                """Causal attention for heads (2cc, 2cc+1), query block qb."""
                qs = qb * 512
                att = ps_att.tile([128, 1024], F32, name="att")
                n_av = [0, 0]

                def av(hh, kb, lo, exp_ap, stop):
                    # two interleaved accumulation groups (hh=0 cols 0:512,
                    # hh=1 cols 512:1024) share the att tile
                    nc.tensor.matmul(
                        att[:, hh * 512 + lo:hh * 512 + 512],
                        va[kb][:, 2 * cc + hh, :], exp_ap,
                        start=(n_av[hh] == 0), stop=stop,
                        skip_group_check=True)
                    n_av[hh] += 1

                # full key blocks: one [128,1024] tile = 1 kb x 2 heads
                for kb in range(4 * qb):
                    s2 = ps_s.tile([128, 1024], F32, name="s2")
                    for hh in range(2):
                        nc.tensor.matmul(
                            s2[:, hh * 512:(hh + 1) * 512],
                            kT[cc][hh * 64:(hh + 1) * 64,
                                   kb * 128:(kb + 1) * 128],
                            qT[cc][hh * 64:(hh + 1) * 64, qs:qs + 512],
                            start=True, stop=True)
                    expT = expp.tile([128, 1024], BF16, name="expT")
                    nc.scalar.activation(out=expT[:], in_=s2[:],
                                         func=AF.Exp, scale=0.125)
                    for hh in range(2):
                        av(hh, kb, 0, expT[:, hh * 512:(hh + 1) * 512],
                           stop=False)
                # diagonal: per jp, one tile per head half (no stale cols)
                for jp in range(2):
                    dt = [ps_s.tile([128, 1024], F32, name="s2")
                          for _ in range(2)]
                    for (m, cb, w) in DIAG[jp]:
                        kb = 4 * qb + m
                        lo = m * 128
                        for hh in range(2):
                            nc.tensor.matmul(
                                dt[hh][:, cb:cb + w],
                                kT[cc][hh * 64:(hh + 1) * 64,
                                       kb * 128:(kb + 1) * 128],
                                qT[cc][hh * 64:(hh + 1) * 64,
                                       qs + lo:qs + 512],
                                start=True, stop=True)
                    tot = sum(w for (_, _, w) in DIAG[jp])
                    expd = [expp.tile([128, 1024], BF16, name="expT")
                            for _ in range(2)]
                    for hh in range(2):
                        nc.scalar.activation(out=expd[hh][:, 0:tot],
                                             in_=dt[hh][:, 0:tot],
                                             func=AF.Exp, scale=0.125)
                    # zero the still-masked triangle (k > q) on DVE
                    for hh in range(2):
                        for (m, cb, w) in DIAG[jp]:
                            nc.vector.tensor_mul(
                                out=expd[hh][:, cb:cb + 128],
                                in0=expd[hh][:, cb:cb + 128], in1=tri01[:])
                    for (m, cb, w) in DIAG[jp]:
                        kb = 4 * qb + m
                        lo = m * 128
                        last = (jp == 1 and m == 3)
                        for hh in range(2):
                            av(hh, kb, lo, expd[hh][:, cb:cb + w], stop=last)
                # normalize: drain att to SBUF first so its PSUM slot frees
                # immediately -- the DRAM round-trip broadcast below can
                # stall >10us when a ReduceScatter hogs the DMA queues,
                # which otherwise blocks the next group's AV accumulation.
                # (reciprocal_approx_fast only works at base partition 0, so
                # the den row is copied down to partition 0 from SBUF.)
                # den -> recip -> round-trip first (it is the latency-
                # critical path to the normalize); araw drain after, so the
                # broadcast DMA is already in flight while it runs.
                den = bcp.tile([1, 1024], F32, name="den")
                nc.vector.tensor_copy(out=den[:], in_=att[64:65, :])
                rec = bcp.tile([1, 1024], F32, name="rec")
                nc.vector.reciprocal_approx_fast(out=rec[:], in_=den[:])
                rec_dr = drp.tile([1, 1024], F32, name="rec_dr")
                nc.sync.dma_start(out=rec_dr[:], in_=rec[:])
                araw = bcp.tile([128, 1024], F32, name="araw")
                nc.vector.tensor_copy(out=araw[:], in_=att[:])
                bc_sb = bcp.tile([64, 1024], F32, name="bc_sb")
                nc.sync.dma_start(out=bc_sb[:], in_=bass.AP(
                    tensor=rec_dr[:].tensor, offset=rec_dr[:].offset,
                    ap=[[0, 64], [1, 1024]]))
                for hh in range(2):
                    nc.vector.tensor_mul(
                        out=ac[cc][hh * 64:(hh + 1) * 64, qs:qs + 512],
                        in0=araw[0:64, hh * 512:(hh + 1) * 512],
                        in1=bc_sb[:, hh * 512:(hh + 1) * 512])

            def wo_partial(qb, dest, row0, ps_s, wop, engs=None):
                """o_part[qb] = ac[:, qb]^T @ wo -> bf16 -> rs_in rows.
                engs: per-DMA trigger queues (default gpsimd). The last
                wo feeds RS B directly; spreading its rs_in writes across
                the then-idle sync/scalar rings gets the collective off
                ~6-10us earlier."""
                if engs is None:
                    engs = [nc.gpsimd] * 4
                for half in range(2):
                    o_ps = ps_s.tile([128, 1024], F32, name="s2")
                    for g in range(2):
                        qt2 = half * 2 + g
                        for cc in range(2):
                            nc.tensor.matmul(
                                o_ps[:, g * 512:(g + 1) * 512],
                                ac[cc][:, qb * 512 + qt2 * 128:
                                       qb * 512 + (qt2 + 1) * 128],
                                wo_sb[cc][:], start=(cc == 0), stop=(cc == 1))
                    o_sb = wop.tile([128, 1024], BF16, name="o_sb")
                    nc.vector.tensor_copy(out=o_sb[:], in_=o_ps[:])
                    for g in range(2):
                        qt2 = half * 2 + g
                        engs[half * 2 + g].dma_start(
                            out=dest[row0 + qt2 * 128:row0 + (qt2 + 1) * 128,
                                     :],
                            in_=o_sb[:, g * 512:(g + 1) * 512])

            RG = [[0, 1], [2, 3], [4, 5], [6, 7]]
            # qb order (0,2,1,3); RS A = [qb0|qb2] launches mid-attention,
            # RS B = [qb1|qb3] at the end.
            wo_args = {0: (rsA_in, 0), 2: (rsA_in, 512),
                       1: (rsB_in, 0), 3: (rsB_in, 512)}
            with tc.tile_pool(name="ps_s", bufs=2, space="PSUM") as ps_s, \
                 tc.tile_pool(name="ps_att", bufs=2, space="PSUM") as ps_att, \
                 tc.tile_pool(name="expp", bufs=3) as expp, \
                 tc.tile_pool(name="bcp", bufs=2) as bcp, \
                 tc.tile_pool(name="wop", bufs=3) as wop:
                order = [0, 2, 1, 3]
                for qi, qb in enumerate(order):
                    attn_pair(0, qb, ps_s, ps_att, expp, bcp)
                    if qi >= 1:
                        pqb = order[qi - 1]
                        dest, row0 = wo_args[pqb]
                        wo_partial(pqb, dest, row0, ps_s, wop)
                    attn_pair(1, qb, ps_s, ps_att, expp, bcp)
                    if qi == 2:
                        nc.gpsimd.collective_compute(
                            "ReduceScatter", ALU.add, replica_groups=RG,
                            ins=[rsA_in[:]], outs=[rsA_out[:]])
                wo_partial(3, rsB_in, 512, ps_s, wop,
                           engs=[nc.scalar, nc.sync, nc.scalar, nc.sync])
                nc.gpsimd.collective_compute(
                    "ReduceScatter", ALU.add, replica_groups=RG,
                    ins=[rsB_in[:]], outs=[rsB_out[:]])
                if DEBUG:
                    nc.sync.dma_start(out=dbg_kt[:, :], in_=kT[0][:])
                    nc.sync.dma_start(out=dbg_qt[:, :], in_=qT[0][:])
                    nc.sync.dma_start(out=dbg_va[:, :],
                                      in_=va[0][:].rearrange(
                                          "p h e -> p (h e)"))
                    for cc in range(2):
                        nc.sync.dma_start(out=dbg_ac[cc, :, :], in_=ac[cc][:])
                    nc.sync.dma_start(out=dbg_rs[0:512, :], in_=rsA_out[:])
                    nc.sync.dma_start(out=dbg_rs[512:1024, :],
                                      in_=rsB_out[:])

            # ---------------- phase 3: Wo + LN1 + FFN + LN2 -------------
            def layer_norm_core(pre, dst):
                stats = lnp.tile([128, 6], F32, name="ln_stats")
                nc.vector.bn_stats(out=stats[:], in_=pre[:])
                mv = lnp.tile([128, 2], F32, name="ln_mv")
                nc.vector.bn_aggr(out=mv[:], in_=stats[:])
                rstd = lnp.tile([128, 1], F32, name="ln_rstd")
                nc.scalar.activation(out=rstd[:], in_=mv[:, 1:2],
                                     func=AF.Sqrt, bias=eps_t[:])
                nc.vector.reciprocal(out=rstd[:], in_=rstd[:])
                nc.vector.tensor_scalar(
                    out=dst, in0=pre[:], scalar1=mv[:, 0:1],
                    scalar2=rstd[:], op0=ALU.subtract, op1=ALU.mult)

            def wo_ln1(half, src, orp):
                """rs_out read + residual + LN1 for rows of this half."""
                for qt2 in range(4):
                    qt = half * 4 + qt2
                    o_rs = orp.tile([128, D], BF16, name="o_rs")
                    nc.sync.dma_start(
                        out=o_rs,
                        in_=src[qt2 * 128:(qt2 + 1) * 128, :])
                    pre = lnp.tile([128, D], F32, name="ln_pre")
                    nc.vector.tensor_add(out=pre[:], in0=o_rs[:],
                                         in1=xbo_sb[qt][:])
                    layer_norm_core(pre, h1[qt][:])

            def transp(half, ps_aux):
                """transpose n1 into h1T, then fold g1/beb2 into h1."""
                for qt2 in range(4):
                    qt = half * 4 + qt2
                    trp = ps_aux.tile([128, D], F32, name="aux")
                    for dc in range(4):
                        nc.tensor.transpose(
                            trp[:, dc * 128:(dc + 1) * 128],
                            h1[qt][:, dc * 128:(dc + 1) * 128], ident[:])
                    nc.vector.tensor_copy(
                        out=h1T[:].rearrange("p (dc c) -> p dc c", dc=4)
                            [:, :, qt * 128:(qt + 1) * 128],
                        in_=trp[:].rearrange("p (dc c) -> p dc c", dc=4))
                for qt2 in range(4):
                    qt = half * 4 + qt2
                    nc.gpsimd.tensor_mul(out=h1[qt][:], in0=h1[qt][:],
                                         in1=g1_bc[:])
                    nc.gpsimd.tensor_add(out=h1[qt][:], in0=h1[qt][:],
                                         in1=beb2_bc[:])

            def ffn1(half, ps_f1, fap, fa_out):
                for fc in range(16):
                    fp_ps = ps_f1.tile([128, 512], F32, name="fp_ps")
                    for dc in range(4):
                        nc.tensor.matmul(
                            fp_ps[:],
                            w1_sb[dc][:, fc * 128:(fc + 1) * 128],
                            h1T[:, dc * OWN + half * 512:
                                dc * OWN + (half + 1) * 512],
                            start=(dc == 0), stop=(dc == 3))
                    fa_t = fap.tile([128, 512], BF16, name=f"fa{fc}")
                    nc.scalar.activation(out=fa_t[:], in_=fp_ps[:],
                                         func=AF.Relu,
                                         bias=b1_col[:, fc:fc + 1])
                    fa_out.append(fa_t)

            def ffn2(half, ps_f2, fa, outp):
                for qt2 in range(4):
                    qt = half * 4 + qt2
                    ff2_ps = ps_f2.tile([128, D], F32, name="ff2")
                    for fc in range(16):
                        nc.tensor.matmul(
                            ff2_ps[:],
                            fa[fc][:, qt2 * 128:(qt2 + 1) * 128],
                            w2_sb[fc][:], start=(fc == 0), stop=(fc == 15))
                    # h1 already holds n1*g1 + (be1+b2)
                    pre = lnp.tile([128, D], F32, name="ln_pre")
                    nc.vector.tensor_add(out=pre[:], in0=ff2_ps[:],
                                         in1=h1[qt][:])
                    out_sb = outp.tile([128, D], F32, name="out_sb")
                    layer_norm_core(pre, out_sb[:])
                    nc.gpsimd.tensor_mul(out=out_sb[:], in0=out_sb[:],
                                         in1=g2_bc[:])
                    nc.gpsimd.tensor_add(out=out_sb[:], in0=out_sb[:],
                                         in1=be2_bc[:])
                    nc.sync.dma_start(
                        out=out_d[qt * 128:(qt + 1) * 128, :], in_=out_sb[:])

            with tc.tile_pool(name="ps_aux", bufs=2, space="PSUM") as ps_aux, \
                 tc.tile_pool(name="ps_f1", bufs=3, space="PSUM") as ps_f1, \
                 tc.tile_pool(name="ps_f2", bufs=2, space="PSUM") as ps_f2, \
                 tc.tile_pool(name="fap", bufs=1) as fap, \
                 tc.tile_pool(name="orp", bufs=3) as orp, \
                 tc.tile_pool(name="outp", bufs=2) as outp:
                fa0, fa1 = [], []
                wo_ln1(0, rsA_out, orp)
                if DEBUG:
                    nc.sync.dma_start(out=dbg_h1[:, :], in_=h1[0][:])
                transp(0, ps_aux)
                ffn1(0, ps_f1, fap, fa0)
                # half-1 LN chain emitted before half-0's LN2 so the DVE
                # order matches readiness (RS B lands before ffn2(0) ends).
                wo_ln1(1, rsB_out, orp)
                ffn2(0, ps_f2, fa0, outp)
                transp(1, ps_aux)
                ffn1(1, ps_f1, fap, fa1)
                ffn2(1, ps_f2, fa1, outp)

    nc.compile()
    return nc


def _get_nc():
    if "nc" not in _CACHE:
        _CACHE["nc"] = _build()
    return _CACHE["nc"]


def _make_in_maps(x, Wq, bq, Wk, bk, Wv, bv, Wo, bo, W1, b1, W2, b2, g1, be1,
                  g2, be2):
    bf = ml_dtypes.bfloat16
    x = np.ascontiguousarray(np.asarray(x, dtype=np.float32))
    Wq, Wk, Wv = (np.asarray(w, np.float32) for w in (Wq, Wk, Wv))
    bo = np.asarray(bo, np.float32)
    g1f = np.asarray(g1, np.float32)
    be1f = np.asarray(be1, np.float32)
    w1f = np.asarray(W1, np.float32)
    w1b = np.ascontiguousarray((g1f[:, None] * w1f).astype(bf))
    b1f = np.asarray(b1, np.float32) + be1f @ w1f
    w2b = np.ascontiguousarray(np.asarray(W2, np.float32).astype(bf))
    wof = np.asarray(Wo, np.float32)
    b1c = np.ascontiguousarray(b1f.reshape(16, 128).T)
    in_maps = []
    for c in range(N_CORES):
        n, s = divmod(c, 2)
        hsel = slice(HH * s, HH * s + HH)
        in_maps.append({
            "xt": np.ascontiguousarray(x[n].T.astype(bf)),
            "xbo": np.ascontiguousarray(x[n, OWN * s:OWN * s + OWN] + bo),
            "wq": np.ascontiguousarray(
                Wq[hsel].transpose(1, 0, 2).reshape(D, E).astype(bf)),
            "wk": np.ascontiguousarray(
                Wk[hsel].transpose(1, 0, 2).reshape(D, E).astype(bf)),
            "wv": np.ascontiguousarray(
                Wv[hsel].transpose(1, 0, 2).reshape(D, E).astype(bf)),
            "bqc": np.ascontiguousarray(
                np.asarray(bq, np.float32)[hsel].reshape(2, 128).T),
            "bkc": np.ascontiguousarray(
                np.asarray(bk, np.float32)[hsel].reshape(2, 128).T),
            "bvr": np.ascontiguousarray(
                np.asarray(bv, np.float32)[hsel]).reshape(1, E),
            "wo": np.ascontiguousarray(wof[E * s:E * s + E].astype(bf)),
            "w1": w1b,
            "b1c": b1c,
            "w2": w2b,
            "b2r": np.asarray(b2, np.float32).reshape(1, D),
            # beb2 rides in the be1r slot: residual bias be1 + b2
            "g1r": np.asarray(g1, np.float32).reshape(1, D),
            "be1r": (be1f + np.asarray(b2, np.float32)).reshape(1, D),
            "g2r": np.asarray(g2, np.float32).reshape(1, D),
            "be2r": np.asarray(be2, np.float32).reshape(1, D),
        })
    return in_maps


def kernel(x, Wq, bq, Wk, bk, Wv, bv, Wo, bo, W1, b1, W2, b2, g1, be1, g2,
           be2, mask=None, **_unused):
    nc = _get_nc()
    in_maps = _make_in_maps(x, Wq, bq, Wk, bk, Wv, bv, Wo, bo, W1, b1, W2, b2,
                            g1, be1, g2, be2)
    res = bass_utils.run_bass_kernel_spmd(
        nc, in_maps, core_ids=list(range(N_CORES)))
    y = np.empty((N, K, D), np.float32)
    for c in range(N_CORES):
        n, s = divmod(c, 2)
        y[n, OWN * s:OWN * s + OWN] = res.results[c]["out"]
    return y


def kernel_timed(x, Wq, bq, Wk, bk, Wv, bv, Wo, bo, W1, b1, W2, b2, g1, be1,
                 g2, be2, mask=None, trace_cores=None, **_unused):
    """Run with NTFF tracing; returns BassKernelResults (exec_time_ns etc)."""
    nc = _get_nc()
    in_maps = _make_in_maps(x, Wq, bq, Wk, bk, Wv, bv, Wo, bo, W1, b1, W2, b2,
                            g1, be1, g2, be2)
    if trace_cores is None:
        trace_cores = list(range(N_CORES))
    return bass_utils.run_bass_kernel_spmd(
        nc, in_maps, core_ids=list(range(N_CORES)), trace=True,
        trace_cores=trace_cores)
